# revision 29
# baseline (speedup 1.0000x reference)
"""minLSTM (2-layer, B=4, S=4096, D=1024) on 8 Trainium2 NeuronCores.

Sharding: core k -> (batch b = k//2, channel half h = k%2).
Each core computes all 4096 timesteps for its batch and its 512 channels.

Math (exact rewrite of the reference; gates stay well inside +-10 for
these input scales so the clamp is a no-op):
  f' = sig(f)/(sig(f)+sig(i)),  i' = 1 - f'
  g  = max(cell + 0.5, sig(cell))
  c_t = f' c_{t-1} + i' g_t
  h   = sig(o) * c

GEMMs (default mode "fp8l1"): layer-1 gates in fp8e4 with
perf_mode=DoubleRow (2 contraction rows per PE cell, ~1.25x over bf16;
weights pre-scaled by 64 on the host, activations rescale by 1/64),
layer-2 in bf16 (fp8 there pushes the error past the 2e-2 gate).
Measured rel err 1.28e-2 vs the fp32 reference.

The 1/(sig(f)+sig(i)) reciprocal runs on the ACT engine's reciprocal
table (~1.2e-5 rel err) as ONE wide op per token block, so the walrus
scheduler cannot scatter it between sigmoids (each occurrence would
cost a 1.3us act-table load).  The scan value term is
btn = (f'-1)*g = -i'*g via one fused scalar_tensor_tensor, undone by
tensor_tensor_scan(mult, subtract); no division anywhere on the DVE.

Engine split per [128 x 512] tile:
  ACT : sig(cell), sig(o) | sig(f), sig(i) | recip(s)  (batched so the
        act table switches exactly twice per token block)
  DVE : g = max(cell + bc, sg)  (fused stt, PSUM read; via cp5 in fp8)
        btn = (a - 1) * g       (fused stt)
        c = scan(a, btn)
  Pool: ssum = sf+si, a = sf*r, h = so*c  (SBUF-only tensor_tensor)
The PE runs gates cell,o for all 4 chunks first, then i,f — every PSUM
bank is drained by an early consumer and the 8 banks cover the
c/o/i/f x 4-chunk working set with double buffering per tag.  Each
block's reciprocal/scan/store tail is emitted one block late so the
act-table switch never delays the next block's sigmoid phase; the
sf/so/g rings are sized (bufs=8) for that extended lifetime.

Both layers' weights prefetch up front on the Activation/GpSimd DMA
queues.  Between the layers, channel-half pairs exchange h1 (bf16) via
pairwise AllGather collectives, one per 512-token block, overlapped
with compute.

Self-contained: hardcodes shapes; only imports the system concourse repo.
"""
import os
import sys

if '/opt/trn_rl_repo' not in sys.path:
    sys.path.insert(0, '/opt/trn_rl_repo')

import numpy as np

B, S, D = 4, 4096, 1024
NCORES = 8
HALF = D // 2           # channels per core: 512
NCHUNK = HALF // 128    # 4 partition chunks of 128 channels
NKT = D // 128          # 8 contraction k-tiles
TBLK = 512              # token block
NBLK = S // TBLK        # 8 token blocks
GCH = 4 * HALF          # gate channels per core: 2048

_CACHE = {}


def _split_multi_waits(nc):
    """This walrus build rejects >1 sync wait per instruction. Hoist extra
    waits onto same-engine NoOps inserted just before; engine-queue program
    order makes this semantically identical."""
    from concourse import mybir
    n = 0
    for fn in nc.m.functions:
        for blk in fn.blocks:
            insts = list(blk.instructions)
            new = []
            changed = False
            for inst in insts:
                si = inst.sync_info
                ow = list(si.on_wait) if si is not None and si.on_wait else []
                if len(ow) > 1:
                    changed = True
                    for w in ow[:-1]:
                        n += 1
                        nop = mybir.InstNoOp(name=f"I-wsplit-{n}", ins=[], outs=[])
                        nop.engine = inst.engine
                        nop.sync_info = mybir.SyncInfo(on_wait=[w], on_update=[])
                        new.append(nop)
                    si.on_wait = [ow[-1]]
                new.append(inst)
            if changed:
                blk.instructions = new
    return n


def _build_nc(mm_mode="fp8l1", sim_local=False):
    import concourse.bass as bass
    import concourse.mybir as mybir
    import concourse.tile as tile

    f32 = mybir.dt.float32
    f8 = mybir.dt.float8e4
    DR = mm_mode == "fp8l1"  # layer-1 fp8 DoubleRow, layer-2 bf16
    fmm = {"f32r": mybir.dt.float32r, "f32": f32, "bf16": mybir.dt.bfloat16,
           "fp8l1": mybir.dt.bfloat16}[mm_mode]
    lmm = [f8 if DR else fmm, fmm]   # per-layer matmul dtype
    fh1 = mybir.dt.bfloat16 if mm_mode in ("bf16", "fp8l1") else f32
    PM = mybir.MatmulPerfMode
    AF = mybir.ActivationFunctionType
    ALU = mybir.AluOpType

    nc = bass.Bass("TRN2", target_bir_lowering=False, debug=False,
                   num_devices=NCORES)

    xT_d = nc.dram_tensor("xT", [D, S], lmm[0], kind="ExternalInput").ap()
    w_d = [nc.dram_tensor(f"w{l}t", [D, GCH], lmm[l], kind="ExternalInput").ap()
           for l in range(2)]
    ba_d = [nc.dram_tensor(f"b{l}a", [128, 16], f32, kind="ExternalInput").ap()
            for l in range(2)]
    bc_d = [nc.dram_tensor(f"b{l}c", [128, 4], f32, kind="ExternalInput").ap()
            for l in range(2)]
    cp_d = [nc.dram_tensor(f"cp{l}", [128, 4], f32, kind="ExternalInput").ap()
            for l in range(2)]
    h2t_d = nc.dram_tensor("h2t", [HALF, S], f32, kind="ExternalOutput").ap()

    with tile.TileContext(nc) as tc:
        with tc.tile_pool(name="wp", bufs=2) as wp, \
             tc.tile_pool(name="xkp", bufs=2) as xkp, \
             tc.tile_pool(name="gp", bufs=2) as gp, \
             tc.tile_pool(name="cp", bufs=1) as cpool, \
             tc.tile_pool(name="psum", bufs=2, space="PSUM") as psum, \
             tc.tile_pool(name="dstage", bufs=2, space="DRAM") as dstage, \
             tc.tile_pool(name="dfull", bufs=8, space="DRAM") as dfull:

            # h1 gathered blocks must persist through layer 2: 8 live tiles
            h1f = [dfull.tile([D, TBLK], fh1, tag="h1f", name=f"h1f{t}")
                   for t in range(NBLK)]

            # Prefetch BOTH layers' weights up front on queues other than
            # Sync (which carries the x loads the first matmuls wait on):
            # layer-1 on the Activation queue, layer-2 on GpSimd.  The DMA
            # engines drain all three descriptor streams concurrently.
            w_ks_all = []
            for l in range(2):
                w_ks = []
                eng = nc.scalar if l == 0 else nc.gpsimd
                # In DR mode the two layers use distinct tags, so each tag
                # only ever holds one tile: bufs=1 (else SBUF overflows).
                wb = 1 if DR else 2
                if DR and l == 0:
                    for k4 in range(NKT // 2):
                        wk = wp.tile([128, 2, GCH], f8, tag=f"Wq{k4}",
                                     name=f"w{l}_{k4}", bufs=wb)
                        eng.dma_start(wk[:],
                                      w_d[l][k4 * 256:(k4 + 1) * 256, :])
                        w_ks.append(wk)
                else:
                    for k in range(NKT):
                        wk = wp.tile([128, GCH], lmm[l], tag=f"Wk{k}",
                                     name=f"w{l}_{k}", bufs=wb)
                        eng.dma_start(wk[:], w_d[l][k * 128:(k + 1) * 128, :])
                        w_ks.append(wk)
                w_ks_all.append(w_ks)

            for l in range(2):
                w_ks = w_ks_all[l]
                ba = cpool.tile([128, 16], f32, tag=f"ba{l}", name=f"ba{l}")
                nc.sync.dma_start(ba[:], ba_d[l][:])
                bc = cpool.tile([128, 4], f32, tag=f"bc{l}", name=f"bc{l}")
                nc.sync.dma_start(bc[:], bc_d[l][:])
                cp = cpool.tile([128, 4], f32, tag=f"cp{l}", name=f"cp{l}")
                nc.sync.dma_start(cp[:], cp_d[l][:])

                carry = [None] * NCHUNK

                def act_recip(out, in_):
                    # The act-table reciprocal measures ~1.2e-5 max rel err
                    # on (9e-5, 2] — emit InstActivation directly since the
                    # bass wrapper refuses Reciprocal.
                    se = nc.scalar
                    se.add_instruction(mybir.InstActivation(
                        name=nc.get_next_instruction_name(),
                        func=AF.Reciprocal,
                        ins=[se.lower_ap(in_),
                             mybir.ImmediateValue(dtype=f32, value=0.0),
                             mybir.ImmediateValue(dtype=f32, value=1.0),
                             mybir.ImmediateValue(dtype=f32, value=0.0)],
                        outs=[se.lower_ap(out)],
                    ))

                def emit_tail(st):
                    """Finish block st: r = 1/s (act table phase), a, btn,
                    scan, h, store + collective.  Emitted one block late so
                    the act-table switch never delays the next block's
                    sigmoid phase (whose DVE g-op gates PSUM bank reuse).
                    recip/a/btn/h are single WIDE [128, 4*TBLK] ops: the
                    walrus scheduler cannot scatter the recip between
                    sigmoids (a table load per occurrence), and one wide
                    Pool/DVE op costs ~1/4 the fixed overhead of four."""
                    t, sf_all, ss_all, g_all, so_all, h1own = st
                    r_all = W2("r", t, bufs=1)
                    act_recip(r_all[:], ss_all[:])
                    a_all = W2("a", t, bufs=1)
                    nc.gpsimd.tensor_tensor(a_all[:], sf_all[:], r_all[:],
                                            ALU.mult)
                    btn_all = W2("bt", t, bufs=1)
                    nc.vector.scalar_tensor_tensor(btn_all[:], a_all[:], 1.0,
                                                   g_all[:], ALU.subtract,
                                                   ALU.mult)
                    c_all = W2("c", t, bufs=1)
                    carr = gp.tile([128, NCHUNK], f32, tag="carr",
                                   name=f"carr{l}_{t}", bufs=2)
                    for j in range(NCHUNK):
                        sl = slice(j * TBLK, (j + 1) * TBLK)
                        init = cp[:, j:j + 1] if t == 0 else carry[j]
                        nc.vector.tensor_tensor_scan(c_all[:, sl],
                                                     a_all[:, sl],
                                                     btn_all[:, sl],
                                                     init, ALU.mult,
                                                     ALU.subtract)
                        # copy the last column out so c_all can be ring-1
                        # (the carry must survive into the next block's tail)
                        nc.vector.tensor_scalar(
                            carr[:, j:j + 1],
                            c_all[:, (j + 1) * TBLK - 1:(j + 1) * TBLK],
                            1.0, None, ALU.mult)
                        carry[j] = carr[:, j:j + 1]
                    hdt = fh1 if l == 0 else f32
                    h_all = W2(f"h{l}", t, dt=hdt, bufs=1)
                    nc.gpsimd.tensor_tensor(h_all[:], so_all[:], c_all[:],
                                            ALU.mult)
                    for j in range(NCHUNK):
                        sl = slice(j * TBLK, (j + 1) * TBLK)
                        if l == 0:
                            nc.sync.dma_start(
                                h1own[j * 128:(j + 1) * 128, :], h_all[:, sl])
                        else:
                            nc.sync.dma_start(
                                h2t_d[j * 128:(j + 1) * 128,
                                      t * TBLK:(t + 1) * TBLK], h_all[:, sl])

                    if l == 0:
                        if sim_local:
                            nc.sync.dma_start(h1f[t][0:HALF, :], h1own[:])
                            nc.sync.dma_start(h1f[t][HALF:D, :], h1own[:])
                        else:
                            nc.gpsimd.collective_compute(
                                "AllGather", ALU.bypass,
                                replica_groups=[[0, 1], [2, 3], [4, 5], [6, 7]],
                                ins=[h1own.opt()],
                                outs=[h1f[t].opt()],
                            )

                def T2(nm, t, j, dt=f32, bufs=2):
                    return gp.tile([128, TBLK], dt, tag=nm,
                                   name=f"{nm}{l}_{t}_{j}", bufs=bufs)

                def W2(nm, t, dt=f32, bufs=2):
                    return gp.tile([128, NCHUNK * TBLK], dt, tag=nm,
                                   name=f"{nm}{l}_{t}", bufs=bufs)

                pending = None
                for t in range(NBLK):
                    xk_ks = []
                    if DR and l == 0:
                        for k4 in range(NKT // 2):
                            xkt = xkp.tile([128, 2, TBLK], f8, tag=f"xq{k4}",
                                           name=f"xq{l}_{t}_{k4}")
                            nc.sync.dma_start(
                                xkt[:], xT_d[k4 * 256:(k4 + 1) * 256,
                                             t * TBLK:(t + 1) * TBLK])
                            xk_ks.append(xkt)
                    else:
                        for k in range(NKT):
                            xkt = xkp.tile([128, TBLK], lmm[l], tag=f"xk{k}",
                                           name=f"xk{l}_{t}_{k}")
                            if l == 0:
                                srcap = xT_d[k * 128:(k + 1) * 128,
                                             t * TBLK:(t + 1) * TBLK]
                            else:
                                srcap = h1f[t][k * 128:(k + 1) * 128, :]
                            nc.sync.dma_start(
                                xkt[:],
                                srcap if srcap.dtype == lmm[l]
                                else srcap.bitcast(lmm[l]))
                            xk_ks.append(xkt)

                    if l == 0:
                        h1own = dstage.tile([HALF, TBLK], fh1, tag="h1own",
                                            name=f"h1own{t}")
                    else:
                        h1own = None

                    def mm(qi, j, tag):
                        ct = qi * NCHUNK + j
                        p = psum.tile([128, TBLK], f32, tag=tag,
                                      name=f"ps{qi}_{l}_{t}_{j}")
                        if DR and l == 0:
                            for k4 in range(NKT // 2):
                                nc.tensor.matmul(
                                    p[:],
                                    w_ks[k4][:, :, ct * 128:(ct + 1) * 128],
                                    xk_ks[k4][:],
                                    start=(k4 == 0), stop=(k4 == NKT // 2 - 1),
                                    perf_mode=PM.DoubleRow)
                        else:
                            for k in range(NKT):
                                nc.tensor.matmul(
                                    p[:],
                                    w_ks[k][:, ct * 128:(ct + 1) * 128],
                                    xk_ks[k][:],
                                    start=(k == 0), stop=(k == NKT - 1))
                        return p
                    sc = 0.015625 if (DR and l == 0) else 1.0

                    # --- phase A: cell,o gates (PSUM drained early) ---
                    # Per-chunk producers write SLICES of wide tiles so the
                    # delayed-tail consumers can be single wide ops.
                    ps_c = [mm(3, j, "pc") for j in range(NCHUNK)]
                    ps_o = [mm(2, j, "po") for j in range(NCHUNK)]
                    sg_all = W2("sg", t, bufs=1)
                    so_all = W2("so", t, bufs=2)
                    g_all = W2("g", t, bufs=2)
                    for j in range(NCHUNK):
                        sl = slice(j * TBLK, (j + 1) * TBLK)
                        nc.scalar.activation(sg_all[:, sl], ps_c[j][:],
                                             AF.Sigmoid,
                                             bias=ba[:, 12 + j:13 + j],
                                             scale=sc)
                        nc.scalar.activation(so_all[:, sl], ps_o[j][:],
                                             AF.Sigmoid,
                                             bias=ba[:, 8 + j:9 + j],
                                             scale=sc)
                    for j in range(NCHUNK):
                        # g = max(cell + bc, sig(cell)) fused; drains ps_c
                        sl = slice(j * TBLK, (j + 1) * TBLK)
                        if DR and l == 0:
                            cp5 = T2("cq", t, j)
                            nc.vector.tensor_scalar(cp5[:], ps_c[j][:], sc,
                                                    bc[:, j:j + 1],
                                                    ALU.mult, ALU.add)
                            nc.vector.tensor_tensor(g_all[:, sl], cp5[:],
                                                    sg_all[:, sl], ALU.max)
                        else:
                            nc.vector.scalar_tensor_tensor(g_all[:, sl],
                                                           ps_c[j][:],
                                                           bc[:, j:j + 1],
                                                           sg_all[:, sl],
                                                           ALU.add, ALU.max)

                    # --- phase B: i,f gates ---
                    ps_i = [mm(0, j, "pi") for j in range(NCHUNK)]
                    ps_f = [mm(1, j, "pf") for j in range(NCHUNK)]
                    sf_all = W2("sf", t, bufs=2)
                    si_all = W2("si", t, bufs=1)
                    for j in range(NCHUNK):
                        sl = slice(j * TBLK, (j + 1) * TBLK)
                        nc.scalar.activation(sf_all[:, sl], ps_f[j][:],
                                             AF.Sigmoid,
                                             bias=ba[:, 4 + j:5 + j],
                                             scale=sc)
                        nc.scalar.activation(si_all[:, sl], ps_i[j][:],
                                             AF.Sigmoid,
                                             bias=ba[:, j:j + 1],
                                             scale=sc)
                    ss_all = W2("ss", t, bufs=2)
                    nc.gpsimd.tensor_tensor(ss_all[:], sf_all[:], si_all[:],
                                            ALU.add)

                    if pending is not None:
                        emit_tail(pending)
                    pending = (t, sf_all, ss_all, g_all, so_all, h1own)
                emit_tail(pending)

    _split_multi_waits(nc)
    return nc


def _shard_inputs(x, W0, b0, W1, b1, c0_prev, c1_prev, mm_mode="fp8l1"):
    import ml_dtypes
    if mm_mode in ("bf16", "fp8l1"):
        mmdt = ml_dtypes.bfloat16
    else:
        mmdt = np.float32
    # fp8l1: layer-1 operands in TRN fp8e4 (max +-240); weights pre-scaled
    # by 64 so they sit in the normal range (the kernel rescales by 1/64
    # inside the activations).
    f8 = ml_dtypes.float8_e4m3
    xdt = [f8 if mm_mode == "fp8l1" else mmdt, mmdt]
    wscale = [np.float32(64.0) if mm_mode == "fp8l1" else np.float32(1.0),
              np.float32(1.0)]
    x = np.asarray(x, dtype=np.float32)
    in_maps = []
    xT = [np.ascontiguousarray(np.clip(x[b].T, -240, 240).astype(xdt[0]))
          for b in range(B)]
    per_layer = []
    for li, (W, bb) in enumerate(((W0, b0), (W1, b1))):
        W = np.asarray(W, dtype=np.float32)
        bb = np.asarray(bb, dtype=np.float32)
        halves = []
        for h in range(2):
            rows = np.concatenate(
                [q * D + h * HALF + np.arange(HALF) for q in range(4)])
            wt = np.ascontiguousarray(
                np.clip(W[rows, :].T * wscale[li], -240, 240)
                .astype(xdt[li]))  # (D, GCH)
            ba = np.ascontiguousarray(bb[rows].reshape(16, 128).T)  # (128,16)
            bc = np.ascontiguousarray(ba[:, 12:16] + np.float32(0.5))
            halves.append((wt, ba, bc))
        per_layer.append(halves)
    cps = []
    for cprev in (c0_prev, c1_prev):
        cprev = np.asarray(cprev, dtype=np.float32)
        halves = []
        for b in range(B):
            row = []
            for h in range(2):
                seg = cprev[b, 0, h * HALF:(h + 1) * HALF]
                row.append(np.ascontiguousarray(seg.reshape(4, 128).T))
            halves.append(row)
        cps.append(halves)
    for k in range(NCORES):
        b, h = k // 2, k % 2
        m = {"xT": xT[b]}
        for l in range(2):
            wt, ba, bc = per_layer[l][h]
            m[f"w{l}t"] = wt
            m[f"b{l}a"] = ba
            m[f"b{l}c"] = bc
            m[f"cp{l}"] = cps[l][b][h]
        in_maps.append(m)
    return in_maps


MM_MODE = os.environ.get("MINLSTM_MM_MODE", "fp8l1")


def _get_nc():
    if "nc" not in _CACHE:
        _CACHE["nc"] = _build_nc(mm_mode=MM_MODE)
    return _CACHE["nc"]


def kernel(x, W0, b0, W1, b1, c0_prev, c1_prev):
    from concourse.bass_utils import run_bass_kernel_spmd

    nc = _get_nc()
    in_maps = _shard_inputs(x, W0, b0, W1, b1, c0_prev, c1_prev, MM_MODE)
    res = run_bass_kernel_spmd(nc, in_maps, list(range(NCORES)))
    out = np.empty((B, S, D), dtype=np.float32)
    for k in range(NCORES):
        b, h = k // 2, k % 2
        out[b, :, h * HALF:(h + 1) * HALF] = res.results[k]["h2t"].T
    return out


# revision 31
# speedup vs baseline: 1.3222x; 1.3222x over previous
"""minLSTM (2-layer, B=4, S=4096, D=1024) on 8 Trainium2 NeuronCores.

Sharding: core k -> (batch b = k//2, channel half h = k%2).
Each core computes all 4096 timesteps for its batch and its 512 channels.

Math (exact rewrite of the reference; gates stay well inside +-10 for
these input scales so the clamp is a no-op):
  f' = sig(f)/(sig(f)+sig(i)),  i' = 1 - f'
  g  = max(cell + 0.5, sig(cell))
  c_t = f' c_{t-1} + i' g_t
  h   = sig(o) * c

GEMMs (default mode "fp8l1"): layer-1 gates in fp8e4 with
perf_mode=DoubleRow (2 contraction rows per PE cell, ~1.25x over bf16;
weights pre-scaled by 64 on the host, activations rescale by 1/64),
layer-2 in bf16 (fp8 there pushes the error past the 2e-2 gate).
Measured rel err 1.28e-2 vs the fp32 reference.

The 1/(sig(f)+sig(i)) reciprocal runs on the ACT engine's reciprocal
table (~1.2e-5 rel err) as ONE wide op per token block, so the walrus
scheduler cannot scatter it between sigmoids (each occurrence would
cost a 1.3us act-table load).  The scan value term is
btn = (f'-1)*g = -i'*g via one fused scalar_tensor_tensor, undone by
tensor_tensor_scan(mult, subtract); no division anywhere on the DVE.

Engine split per [128 x 512] tile:
  ACT : sig(cell), sig(o) | sig(f), sig(i) | recip(s)  (batched so the
        act table switches exactly twice per token block)
  DVE : g = max(cell + bc, sg)  (fused stt, PSUM read; via cp5 in fp8)
        btn = (a - 1) * g       (fused stt)
        c = scan(a, btn)
  Pool: ssum = sf+si, a = sf*r, h = so*c  (SBUF-only tensor_tensor)
The PE runs gates cell,o for all 4 chunks first, then i,f — every PSUM
bank is drained by an early consumer and the 8 banks cover the
c/o/i/f x 4-chunk working set with double buffering per tag.  Each
block's reciprocal/scan/store tail is emitted one block late so the
act-table switch never delays the next block's sigmoid phase; the
sf/so/g rings are sized (bufs=8) for that extended lifetime.

Both layers' weights prefetch up front on the Activation/GpSimd DMA
queues.  Between the layers, channel-half pairs exchange h1 (bf16) via
pairwise AllGather collectives, one per 512-token block, overlapped
with compute.

Self-contained: hardcodes shapes; only imports the system concourse repo.
"""
import os
import sys

if '/opt/trn_rl_repo' not in sys.path:
    sys.path.insert(0, '/opt/trn_rl_repo')

import numpy as np

B, S, D = 4, 4096, 1024
NCORES = 8
HALF = D // 2           # channels per core: 512
NCHUNK = HALF // 128    # 4 partition chunks of 128 channels
NKT = D // 128          # 8 contraction k-tiles
TBLK = 512              # token block
NBLK = S // TBLK        # 8 token blocks
GCH = 4 * HALF          # gate channels per core: 2048

_CACHE = {}


def _split_multi_waits(nc):
    """This walrus build rejects >1 sync wait per instruction. Hoist extra
    waits onto same-engine NoOps inserted just before; engine-queue program
    order makes this semantically identical."""
    from concourse import mybir
    n = 0
    for fn in nc.m.functions:
        for blk in fn.blocks:
            insts = list(blk.instructions)
            new = []
            changed = False
            for inst in insts:
                si = inst.sync_info
                ow = list(si.on_wait) if si is not None and si.on_wait else []
                if len(ow) > 1:
                    changed = True
                    for w in ow[:-1]:
                        n += 1
                        nop = mybir.InstNoOp(name=f"I-wsplit-{n}", ins=[], outs=[])
                        nop.engine = inst.engine
                        nop.sync_info = mybir.SyncInfo(on_wait=[w], on_update=[])
                        new.append(nop)
                    si.on_wait = [ow[-1]]
                new.append(inst)
            if changed:
                blk.instructions = new
    return n


def _build_nc(mm_mode="fp8l1", sim_local=False):
    import concourse.bass as bass
    import concourse.mybir as mybir
    import concourse.tile as tile

    f32 = mybir.dt.float32
    f8 = mybir.dt.float8e4
    DR = mm_mode == "fp8l1"  # layer-1 fp8 DoubleRow, layer-2 bf16
    fmm = {"f32r": mybir.dt.float32r, "f32": f32, "bf16": mybir.dt.bfloat16,
           "fp8l1": mybir.dt.bfloat16}[mm_mode]
    lmm = [f8 if DR else fmm, fmm]   # per-layer matmul dtype
    fh1 = mybir.dt.bfloat16 if mm_mode in ("bf16", "fp8l1") else f32
    PM = mybir.MatmulPerfMode
    AF = mybir.ActivationFunctionType
    ALU = mybir.AluOpType

    nc = bass.Bass("TRN2", target_bir_lowering=False, debug=False,
                   num_devices=NCORES)

    xT_d = nc.dram_tensor("xT", [D, S], lmm[0], kind="ExternalInput").ap()
    w_d = [nc.dram_tensor(f"w{l}t", [D, GCH], lmm[l], kind="ExternalInput").ap()
           for l in range(2)]
    ba_d = [nc.dram_tensor(f"b{l}a", [128, 16], f32, kind="ExternalInput").ap()
            for l in range(2)]
    bc_d = [nc.dram_tensor(f"b{l}c", [128, 4], f32, kind="ExternalInput").ap()
            for l in range(2)]
    cp_d = [nc.dram_tensor(f"cp{l}", [128, 4], f32, kind="ExternalInput").ap()
            for l in range(2)]
    h2t_d = nc.dram_tensor("h2t", [HALF, S], f32, kind="ExternalOutput").ap()

    with tile.TileContext(nc) as tc:
        with tc.tile_pool(name="wp", bufs=2) as wp, \
             tc.tile_pool(name="xkp", bufs=2) as xkp, \
             tc.tile_pool(name="gp", bufs=2) as gp, \
             tc.tile_pool(name="cp", bufs=1) as cpool, \
             tc.tile_pool(name="psum", bufs=2, space="PSUM") as psum, \
             tc.tile_pool(name="dstage", bufs=2, space="DRAM") as dstage, \
             tc.tile_pool(name="dfull", bufs=8, space="DRAM") as dfull:

            # h1 gathered blocks must persist through layer 2: 8 live tiles
            h1f = [dfull.tile([D, TBLK], fh1, tag="h1f", name=f"h1f{t}")
                   for t in range(NBLK)]

            # Prefetch BOTH layers' weights up front on queues other than
            # Sync (which carries the x loads the first matmuls wait on):
            # layer-1 on the Activation queue, layer-2 on GpSimd.  The DMA
            # engines drain all three descriptor streams concurrently.
            w_ks_all = []
            for l in range(2):
                w_ks = []
                eng = nc.scalar if l == 0 else nc.gpsimd
                # In DR mode the two layers use distinct tags, so each tag
                # only ever holds one tile: bufs=1 (else SBUF overflows).
                wb = 1 if DR else 2
                if DR and l == 0:
                    for k4 in range(NKT // 2):
                        wk = wp.tile([128, 2, GCH], f8, tag=f"Wq{k4}",
                                     name=f"w{l}_{k4}", bufs=wb)
                        eng.dma_start(wk[:],
                                      w_d[l][k4 * 256:(k4 + 1) * 256, :])
                        w_ks.append(wk)
                else:
                    for k in range(NKT):
                        wk = wp.tile([128, GCH], lmm[l], tag=f"Wk{k}",
                                     name=f"w{l}_{k}", bufs=wb)
                        eng.dma_start(wk[:], w_d[l][k * 128:(k + 1) * 128, :])
                        w_ks.append(wk)
                w_ks_all.append(w_ks)

            for l in range(2):
                w_ks = w_ks_all[l]
                ba = cpool.tile([128, 16], f32, tag=f"ba{l}", name=f"ba{l}")
                nc.sync.dma_start(ba[:], ba_d[l][:])
                bc = cpool.tile([128, 4], f32, tag=f"bc{l}", name=f"bc{l}")
                nc.sync.dma_start(bc[:], bc_d[l][:])
                cp = cpool.tile([128, 4], f32, tag=f"cp{l}", name=f"cp{l}")
                nc.sync.dma_start(cp[:], cp_d[l][:])

                carry = [None] * NCHUNK

                def act_recip(out, in_):
                    # The act-table reciprocal measures ~1.2e-5 max rel err
                    # on (9e-5, 2] — emit InstActivation directly since the
                    # bass wrapper refuses Reciprocal.
                    se = nc.scalar
                    se.add_instruction(mybir.InstActivation(
                        name=nc.get_next_instruction_name(),
                        func=AF.Reciprocal,
                        ins=[se.lower_ap(in_),
                             mybir.ImmediateValue(dtype=f32, value=0.0),
                             mybir.ImmediateValue(dtype=f32, value=1.0),
                             mybir.ImmediateValue(dtype=f32, value=0.0)],
                        outs=[se.lower_ap(out)],
                    ))

                def emit_tail(st):
                    """Finish block st: r = 1/s (act table phase), a, btn,
                    scan, h, store + collective.  Emitted one block late so
                    the act-table switch never delays the next block's
                    sigmoid phase (whose DVE g-op gates PSUM bank reuse).
                    The reciprocal is ONE wide op over all 4 chunks so the
                    walrus scheduler cannot scatter it between sigmoids
                    (which would add an act-table load per occurrence)."""
                    t, sfs, ss_all, gs, sos, h1own = st
                    r_all = gp.tile([128, NCHUNK * TBLK], f32, tag="r",
                                    name=f"r{l}_{t}", bufs=2)
                    rs = [r_all[:, j * TBLK:(j + 1) * TBLK]
                          for j in range(NCHUNK)]
                    act_recip(r_all[:], ss_all[:])
                    for j in range(NCHUNK):
                        a = T2("a", t, j)
                        nc.gpsimd.tensor_tensor(a[:], sfs[j][:], rs[j],
                                                ALU.mult)
                        btn = T2("bt", t, j)
                        nc.vector.scalar_tensor_tensor(btn[:], a[:], 1.0,
                                                       gs[j][:], ALU.subtract,
                                                       ALU.mult)
                        c = T2(f"c{j}", t, j)
                        init = cp[:, j:j + 1] if t == 0 else carry[j]
                        nc.vector.tensor_tensor_scan(c[:], a[:], btn[:],
                                                     init, ALU.mult,
                                                     ALU.subtract)
                        carry[j] = c[:, TBLK - 1:TBLK]
                        hdt = fh1 if l == 0 else f32
                        h = T2(f"h{l}", t, j, hdt)
                        nc.gpsimd.tensor_tensor(h[:], sos[j][:], c[:], ALU.mult)

                        if l == 0:
                            nc.sync.dma_start(
                                h1own[j * 128:(j + 1) * 128, :], h[:])
                        else:
                            nc.sync.dma_start(
                                h2t_d[j * 128:(j + 1) * 128,
                                      t * TBLK:(t + 1) * TBLK], h[:])

                    if l == 0:
                        if sim_local:
                            nc.sync.dma_start(h1f[t][0:HALF, :], h1own[:])
                            nc.sync.dma_start(h1f[t][HALF:D, :], h1own[:])
                        else:
                            nc.gpsimd.collective_compute(
                                "AllGather", ALU.bypass,
                                replica_groups=[[0, 1], [2, 3], [4, 5], [6, 7]],
                                ins=[h1own.opt()],
                                outs=[h1f[t].opt()],
                            )

                def T2(nm, t, j, dt=f32, bufs=2):
                    return gp.tile([128, TBLK], dt, tag=nm,
                                   name=f"{nm}{l}_{t}_{j}", bufs=bufs)

                def W2(nm, t, dt=f32, bufs=2):
                    return gp.tile([128, NCHUNK * TBLK], dt, tag=nm,
                                   name=f"{nm}{l}_{t}", bufs=bufs)

                pending = None
                for t in range(NBLK):
                    xk_ks = []
                    if DR and l == 0:
                        for k4 in range(NKT // 2):
                            xkt = xkp.tile([128, 2, TBLK], f8, tag=f"xq{k4}",
                                           name=f"xq{l}_{t}_{k4}")
                            nc.sync.dma_start(
                                xkt[:], xT_d[k4 * 256:(k4 + 1) * 256,
                                             t * TBLK:(t + 1) * TBLK])
                            xk_ks.append(xkt)
                    else:
                        for k in range(NKT):
                            xkt = xkp.tile([128, TBLK], lmm[l], tag=f"xk{k}",
                                           name=f"xk{l}_{t}_{k}")
                            if l == 0:
                                srcap = xT_d[k * 128:(k + 1) * 128,
                                             t * TBLK:(t + 1) * TBLK]
                            else:
                                srcap = h1f[t][k * 128:(k + 1) * 128, :]
                            nc.sync.dma_start(
                                xkt[:],
                                srcap if srcap.dtype == lmm[l]
                                else srcap.bitcast(lmm[l]))
                            xk_ks.append(xkt)

                    if l == 0:
                        h1own = dstage.tile([HALF, TBLK], fh1, tag="h1own",
                                            name=f"h1own{t}")
                    else:
                        h1own = None

                    def mm(qi, j, tag):
                        ct = qi * NCHUNK + j
                        p = psum.tile([128, TBLK], f32, tag=tag,
                                      name=f"ps{qi}_{l}_{t}_{j}")
                        if DR and l == 0:
                            for k4 in range(NKT // 2):
                                nc.tensor.matmul(
                                    p[:],
                                    w_ks[k4][:, :, ct * 128:(ct + 1) * 128],
                                    xk_ks[k4][:],
                                    start=(k4 == 0), stop=(k4 == NKT // 2 - 1),
                                    perf_mode=PM.DoubleRow)
                        else:
                            for k in range(NKT):
                                nc.tensor.matmul(
                                    p[:],
                                    w_ks[k][:, ct * 128:(ct + 1) * 128],
                                    xk_ks[k][:],
                                    start=(k == 0), stop=(k == NKT - 1))
                        return p
                    sc = 0.015625 if (DR and l == 0) else 1.0

                    # --- phase A: cell,o gates (PSUM drained early) ---
                    ps_c = [mm(3, j, "pc") for j in range(NCHUNK)]
                    ps_o = [mm(2, j, "po") for j in range(NCHUNK)]
                    sgs, sos, gs = [], [], []
                    for j in range(NCHUNK):
                        sg = T2("sg", t, j, bufs=4)
                        nc.scalar.activation(sg[:], ps_c[j][:], AF.Sigmoid,
                                             bias=ba[:, 12 + j:13 + j],
                                             scale=sc)
                        so = T2("so", t, j, bufs=8)
                        nc.scalar.activation(so[:], ps_o[j][:], AF.Sigmoid,
                                             bias=ba[:, 8 + j:9 + j],
                                             scale=sc)
                        sgs.append(sg)
                        sos.append(so)
                    for j in range(NCHUNK):
                        # g = max(cell + bc, sig(cell)) fused; drains ps_c
                        g = T2("g", t, j, bufs=8)
                        if DR and l == 0:
                            cp5 = T2("cq", t, j)
                            nc.vector.tensor_scalar(cp5[:], ps_c[j][:], sc,
                                                    bc[:, j:j + 1],
                                                    ALU.mult, ALU.add)
                            nc.vector.tensor_tensor(g[:], cp5[:], sgs[j][:],
                                                    ALU.max)
                        else:
                            nc.vector.scalar_tensor_tensor(g[:], ps_c[j][:],
                                                           bc[:, j:j + 1],
                                                           sgs[j][:],
                                                           ALU.add, ALU.max)
                        gs.append(g)

                    # --- phase B: i,f gates ---
                    ps_i = [mm(0, j, "pi") for j in range(NCHUNK)]
                    ps_f = [mm(1, j, "pf") for j in range(NCHUNK)]
                    sfs, sis = [], []
                    for j in range(NCHUNK):
                        sf = T2("sf", t, j, bufs=8)
                        nc.scalar.activation(sf[:], ps_f[j][:], AF.Sigmoid,
                                             bias=ba[:, 4 + j:5 + j],
                                             scale=sc)
                        si = T2("si", t, j, bufs=4)
                        nc.scalar.activation(si[:], ps_i[j][:], AF.Sigmoid,
                                             bias=ba[:, j:j + 1],
                                             scale=sc)
                        sfs.append(sf)
                        sis.append(si)
                    ss_all = gp.tile([128, NCHUNK * TBLK], f32, tag="ss",
                                     name=f"ss{l}_{t}", bufs=2)
                    for j in range(NCHUNK):
                        nc.gpsimd.tensor_tensor(
                            ss_all[:, j * TBLK:(j + 1) * TBLK],
                            sfs[j][:], sis[j][:], ALU.add)

                    if pending is not None:
                        emit_tail(pending)
                    pending = (t, sfs, ss_all, gs, sos, h1own)
                emit_tail(pending)

    _split_multi_waits(nc)
    return nc


def _shard_inputs(x, W0, b0, W1, b1, c0_prev, c1_prev, mm_mode="fp8l1"):
    import ml_dtypes
    if mm_mode in ("bf16", "fp8l1"):
        mmdt = ml_dtypes.bfloat16
    else:
        mmdt = np.float32
    # fp8l1: layer-1 operands in TRN fp8e4 (max +-240); weights pre-scaled
    # by 64 so they sit in the normal range (the kernel rescales by 1/64
    # inside the activations).
    f8 = ml_dtypes.float8_e4m3
    xdt = [f8 if mm_mode == "fp8l1" else mmdt, mmdt]
    wscale = [np.float32(64.0) if mm_mode == "fp8l1" else np.float32(1.0),
              np.float32(1.0)]
    x = np.asarray(x, dtype=np.float32)
    in_maps = []
    xT = [np.ascontiguousarray(np.clip(x[b].T, -240, 240).astype(xdt[0]))
          for b in range(B)]
    per_layer = []
    for li, (W, bb) in enumerate(((W0, b0), (W1, b1))):
        W = np.asarray(W, dtype=np.float32)
        bb = np.asarray(bb, dtype=np.float32)
        halves = []
        for h in range(2):
            rows = np.concatenate(
                [q * D + h * HALF + np.arange(HALF) for q in range(4)])
            wt = np.ascontiguousarray(
                np.clip(W[rows, :].T * wscale[li], -240, 240)
                .astype(xdt[li]))  # (D, GCH)
            ba = np.ascontiguousarray(bb[rows].reshape(16, 128).T)  # (128,16)
            bc = np.ascontiguousarray(ba[:, 12:16] + np.float32(0.5))
            halves.append((wt, ba, bc))
        per_layer.append(halves)
    cps = []
    for cprev in (c0_prev, c1_prev):
        cprev = np.asarray(cprev, dtype=np.float32)
        halves = []
        for b in range(B):
            row = []
            for h in range(2):
                seg = cprev[b, 0, h * HALF:(h + 1) * HALF]
                row.append(np.ascontiguousarray(seg.reshape(4, 128).T))
            halves.append(row)
        cps.append(halves)
    for k in range(NCORES):
        b, h = k // 2, k % 2
        m = {"xT": xT[b]}
        for l in range(2):
            wt, ba, bc = per_layer[l][h]
            m[f"w{l}t"] = wt
            m[f"b{l}a"] = ba
            m[f"b{l}c"] = bc
            m[f"cp{l}"] = cps[l][b][h]
        in_maps.append(m)
    return in_maps


MM_MODE = os.environ.get("MINLSTM_MM_MODE", "fp8l1")


def _get_nc():
    if "nc" not in _CACHE:
        _CACHE["nc"] = _build_nc(mm_mode=MM_MODE)
    return _CACHE["nc"]


def kernel(x, W0, b0, W1, b1, c0_prev, c1_prev):
    from concourse.bass_utils import run_bass_kernel_spmd

    nc = _get_nc()
    in_maps = _shard_inputs(x, W0, b0, W1, b1, c0_prev, c1_prev, MM_MODE)
    res = run_bass_kernel_spmd(nc, in_maps, list(range(NCORES)))
    out = np.empty((B, S, D), dtype=np.float32)
    for k in range(NCORES):
        b, h = k // 2, k % 2
        out[b, :, h * HALF:(h + 1) * HALF] = res.results[k]["h2t"].T
    return out


# revision 32
# speedup vs baseline: 1.3691x; 1.0354x over previous
"""minLSTM (2-layer, B=4, S=4096, D=1024) on 8 Trainium2 NeuronCores.

Sharding: core k -> (batch b = k//2, channel half h = k%2).
Each core computes all 4096 timesteps for its batch and its 512 channels.

Math (exact rewrite of the reference; gates stay well inside +-10 for
these input scales so the clamp is a no-op):
  f' = sig(f)/(sig(f)+sig(i)),  i' = 1 - f'
  g  = max(cell + 0.5, sig(cell))
  c_t = f' c_{t-1} + i' g_t
  h   = sig(o) * c

GEMMs (default mode "fp8l1"): layer-1 gates in fp8e4 with
perf_mode=DoubleRow (2 contraction rows per PE cell, ~1.25x over bf16;
weights pre-scaled by 64 on the host, activations rescale by 1/64),
layer-2 in bf16 (fp8 there pushes the error past the 2e-2 gate).
Measured rel err 1.28e-2 vs the fp32 reference.

The 1/(sig(f)+sig(i)) reciprocal runs on the ACT engine's reciprocal
table (~1.2e-5 rel err) as ONE wide op per token block, so the walrus
scheduler cannot scatter it between sigmoids (each occurrence would
cost a 1.3us act-table load).  The scan value term is
btn = (f'-1)*g = -i'*g via one fused scalar_tensor_tensor, undone by
tensor_tensor_scan(mult, subtract); no division anywhere on the DVE.

Engine split per [128 x 512] tile:
  ACT : sig(cell), sig(o) | sig(f), sig(i) | recip(s)  (batched so the
        act table switches exactly twice per token block)
  DVE : g = max(cell + bc, sg)  (fused stt, PSUM read; via cp5 in fp8)
        btn = (a - 1) * g       (fused stt)
        c = scan(a, btn)
  Pool: ssum = sf+si, a = sf*r, h = so*c  (SBUF-only tensor_tensor)
The PE runs gates cell,o for all 4 chunks first, then i,f — every PSUM
bank is drained by an early consumer and the 8 banks cover the
c/o/i/f x 4-chunk working set with double buffering per tag.  Each
block's reciprocal/scan/store tail is emitted one block late so the
act-table switch never delays the next block's sigmoid phase; the
sf/so/g rings are sized (bufs=8) for that extended lifetime.

Both layers' weights prefetch up front on the Activation/GpSimd DMA
queues.  Between the layers, channel-half pairs exchange h1 (bf16) via
pairwise AllGather collectives, one per 512-token block, overlapped
with compute.

Self-contained: hardcodes shapes; only imports the system concourse repo.
"""
import os
import sys

if '/opt/trn_rl_repo' not in sys.path:
    sys.path.insert(0, '/opt/trn_rl_repo')

import numpy as np

B, S, D = 4, 4096, 1024
NCORES = 8
HALF = D // 2           # channels per core: 512
NCHUNK = HALF // 128    # 4 partition chunks of 128 channels
NKT = D // 128          # 8 contraction k-tiles
TBLK = 512              # token block
NBLK = S // TBLK        # 8 token blocks
GCH = 4 * HALF          # gate channels per core: 2048

_CACHE = {}


def _split_multi_waits(nc):
    """This walrus build rejects >1 sync wait per instruction. Hoist extra
    waits onto same-engine NoOps inserted just before; engine-queue program
    order makes this semantically identical."""
    from concourse import mybir
    n = 0
    for fn in nc.m.functions:
        for blk in fn.blocks:
            insts = list(blk.instructions)
            new = []
            changed = False
            for inst in insts:
                si = inst.sync_info
                ow = list(si.on_wait) if si is not None and si.on_wait else []
                if len(ow) > 1:
                    changed = True
                    for w in ow[:-1]:
                        n += 1
                        nop = mybir.InstNoOp(name=f"I-wsplit-{n}", ins=[], outs=[])
                        nop.engine = inst.engine
                        nop.sync_info = mybir.SyncInfo(on_wait=[w], on_update=[])
                        new.append(nop)
                    si.on_wait = [ow[-1]]
                new.append(inst)
            if changed:
                blk.instructions = new
    return n


def _build_nc(mm_mode="fp8l1", sim_local=False):
    import concourse.bass as bass
    import concourse.mybir as mybir
    import concourse.tile as tile

    f32 = mybir.dt.float32
    f8 = mybir.dt.float8e4
    DR = mm_mode == "fp8l1"  # layer-1 fp8 DoubleRow, layer-2 bf16
    fmm = {"f32r": mybir.dt.float32r, "f32": f32, "bf16": mybir.dt.bfloat16,
           "fp8l1": mybir.dt.bfloat16}[mm_mode]
    lmm = [f8 if DR else fmm, fmm]   # per-layer matmul dtype
    fh1 = mybir.dt.bfloat16 if mm_mode in ("bf16", "fp8l1") else f32
    PM = mybir.MatmulPerfMode
    AF = mybir.ActivationFunctionType
    ALU = mybir.AluOpType

    nc = bass.Bass("TRN2", target_bir_lowering=False, debug=False,
                   num_devices=NCORES)

    xT_d = nc.dram_tensor("xT", [D, S], lmm[0], kind="ExternalInput").ap()
    w_d = [nc.dram_tensor(f"w{l}t", [D, GCH], lmm[l], kind="ExternalInput").ap()
           for l in range(2)]
    ba_d = [nc.dram_tensor(f"b{l}a", [128, 16], f32, kind="ExternalInput").ap()
            for l in range(2)]
    bc_d = [nc.dram_tensor(f"b{l}c", [128, 4], f32, kind="ExternalInput").ap()
            for l in range(2)]
    cp_d = [nc.dram_tensor(f"cp{l}", [128, 4], f32, kind="ExternalInput").ap()
            for l in range(2)]
    h2t_d = nc.dram_tensor("h2t", [HALF, S], f32, kind="ExternalOutput").ap()

    with tile.TileContext(nc) as tc:
        with tc.tile_pool(name="wp", bufs=2) as wp, \
             tc.tile_pool(name="xkp", bufs=2) as xkp, \
             tc.tile_pool(name="gp", bufs=2) as gp, \
             tc.tile_pool(name="cp", bufs=1) as cpool, \
             tc.tile_pool(name="psum", bufs=2, space="PSUM") as psum, \
             tc.tile_pool(name="dstage", bufs=2, space="DRAM") as dstage, \
             tc.tile_pool(name="dfull", bufs=8, space="DRAM") as dfull:

            # h1 gathered blocks must persist through layer 2: 8 live tiles
            h1f = [dfull.tile([D, TBLK], fh1, tag="h1f", name=f"h1f{t}")
                   for t in range(NBLK)]

            # Prefetch BOTH layers' weights up front on queues other than
            # Sync (which carries the x loads the first matmuls wait on):
            # layer-1 on the Activation queue, layer-2 on GpSimd.  The DMA
            # engines drain all three descriptor streams concurrently.
            w_ks_all = []
            for l in range(2):
                w_ks = []
                eng = nc.scalar if l == 0 else nc.gpsimd
                # In DR mode the two layers use distinct tags, so each tag
                # only ever holds one tile: bufs=1 (else SBUF overflows).
                wb = 1 if DR else 2
                if DR and l == 0:
                    for k4 in range(NKT // 2):
                        wk = wp.tile([128, 2, GCH], f8, tag=f"Wq{k4}",
                                     name=f"w{l}_{k4}", bufs=wb)
                        eng.dma_start(wk[:],
                                      w_d[l][k4 * 256:(k4 + 1) * 256, :])
                        w_ks.append(wk)
                else:
                    for k in range(NKT):
                        wk = wp.tile([128, GCH], lmm[l], tag=f"Wk{k}",
                                     name=f"w{l}_{k}", bufs=wb)
                        eng.dma_start(wk[:], w_d[l][k * 128:(k + 1) * 128, :])
                        w_ks.append(wk)
                w_ks_all.append(w_ks)

            for l in range(2):
                w_ks = w_ks_all[l]
                ba = cpool.tile([128, 16], f32, tag=f"ba{l}", name=f"ba{l}")
                nc.sync.dma_start(ba[:], ba_d[l][:])
                bc = cpool.tile([128, 4], f32, tag=f"bc{l}", name=f"bc{l}")
                nc.sync.dma_start(bc[:], bc_d[l][:])
                cp = cpool.tile([128, 4], f32, tag=f"cp{l}", name=f"cp{l}")
                nc.sync.dma_start(cp[:], cp_d[l][:])

                carry = [None] * NCHUNK

                def act_recip(out, in_):
                    # The act-table reciprocal measures ~1.2e-5 max rel err
                    # on (9e-5, 2] — emit InstActivation directly since the
                    # bass wrapper refuses Reciprocal.
                    se = nc.scalar
                    se.add_instruction(mybir.InstActivation(
                        name=nc.get_next_instruction_name(),
                        func=AF.Reciprocal,
                        ins=[se.lower_ap(in_),
                             mybir.ImmediateValue(dtype=f32, value=0.0),
                             mybir.ImmediateValue(dtype=f32, value=1.0),
                             mybir.ImmediateValue(dtype=f32, value=0.0)],
                        outs=[se.lower_ap(out)],
                    ))

                def emit_tail(st):
                    """Finish block st: r = 1/s (act table phase), a, btn,
                    scan, h, store + collective.  Emitted one block late so
                    the act-table switch never delays the next block's
                    sigmoid phase (whose DVE g-op gates PSUM bank reuse).
                    The reciprocal is ONE wide op over all 4 chunks so the
                    walrus scheduler cannot scatter it between sigmoids
                    (which would add an act-table load per occurrence)."""
                    t, sfs, ss_all, gs, sos, h1own = st
                    r_all = gp.tile([128, NCHUNK * TBLK], f32, tag="r",
                                    name=f"r{l}_{t}", bufs=2)
                    rs = [r_all[:, j * TBLK:(j + 1) * TBLK]
                          for j in range(NCHUNK)]
                    act_recip(r_all[:], ss_all[:])
                    for j in range(NCHUNK):
                        a = T2("a", t, j)
                        nc.gpsimd.tensor_tensor(a[:], sfs[j][:], rs[j],
                                                ALU.mult)
                        btn = T2("bt", t, j)
                        nc.vector.scalar_tensor_tensor(btn[:], a[:], 1.0,
                                                       gs[j][:], ALU.subtract,
                                                       ALU.mult)
                        c = T2(f"c{j}", t, j)
                        init = cp[:, j:j + 1] if t == 0 else carry[j]
                        nc.vector.tensor_tensor_scan(c[:], a[:], btn[:],
                                                     init, ALU.mult,
                                                     ALU.subtract)
                        carry[j] = c[:, TBLK - 1:TBLK]
                        hdt = fh1 if l == 0 else f32
                        h = T2(f"h{l}", t, j, hdt)
                        nc.gpsimd.tensor_tensor(h[:], sos[j][:], c[:], ALU.mult)

                        if l == 0:
                            nc.sync.dma_start(
                                h1own[j * 128:(j + 1) * 128, :], h[:])
                        else:
                            nc.sync.dma_start(
                                h2t_d[j * 128:(j + 1) * 128,
                                      t * TBLK:(t + 1) * TBLK], h[:])

                    if l == 0:
                        if sim_local:
                            nc.sync.dma_start(h1f[t][0:HALF, :], h1own[:])
                            nc.sync.dma_start(h1f[t][HALF:D, :], h1own[:])
                        else:
                            nc.gpsimd.collective_compute(
                                "AllGather", ALU.bypass,
                                replica_groups=[[0, 1], [2, 3], [4, 5], [6, 7]],
                                ins=[h1own.opt()],
                                outs=[h1f[t].opt()],
                            )

                def T2(nm, t, j, dt=f32, bufs=2):
                    return gp.tile([128, TBLK], dt, tag=nm,
                                   name=f"{nm}{l}_{t}_{j}", bufs=bufs)

                def W2(nm, t, dt=f32, bufs=2):
                    return gp.tile([128, NCHUNK * TBLK], dt, tag=nm,
                                   name=f"{nm}{l}_{t}", bufs=bufs)

                pending = None
                for t in range(NBLK):
                    xk_ks = []
                    if DR and l == 0:
                        for k4 in range(NKT // 2):
                            xkt = xkp.tile([128, 2, TBLK], f8, tag=f"xq{k4}",
                                           name=f"xq{l}_{t}_{k4}")
                            nc.sync.dma_start(
                                xkt[:], xT_d[k4 * 256:(k4 + 1) * 256,
                                             t * TBLK:(t + 1) * TBLK])
                            xk_ks.append(xkt)
                    else:
                        for k in range(NKT):
                            xkt = xkp.tile([128, TBLK], lmm[l], tag=f"xk{k}",
                                           name=f"xk{l}_{t}_{k}")
                            if l == 0:
                                srcap = xT_d[k * 128:(k + 1) * 128,
                                             t * TBLK:(t + 1) * TBLK]
                            else:
                                srcap = h1f[t][k * 128:(k + 1) * 128, :]
                            nc.sync.dma_start(
                                xkt[:],
                                srcap if srcap.dtype == lmm[l]
                                else srcap.bitcast(lmm[l]))
                            xk_ks.append(xkt)

                    if l == 0:
                        h1own = dstage.tile([HALF, TBLK], fh1, tag="h1own",
                                            name=f"h1own{t}")
                    else:
                        h1own = None

                    def mm(qi, j, tag):
                        ct = qi * NCHUNK + j
                        p = psum.tile([128, TBLK], f32, tag=tag,
                                      name=f"ps{qi}_{l}_{t}_{j}")
                        if DR and l == 0:
                            for k4 in range(NKT // 2):
                                nc.tensor.matmul(
                                    p[:],
                                    w_ks[k4][:, :, ct * 128:(ct + 1) * 128],
                                    xk_ks[k4][:],
                                    start=(k4 == 0), stop=(k4 == NKT // 2 - 1),
                                    perf_mode=PM.DoubleRow)
                        else:
                            for k in range(NKT):
                                nc.tensor.matmul(
                                    p[:],
                                    w_ks[k][:, ct * 128:(ct + 1) * 128],
                                    xk_ks[k][:],
                                    start=(k == 0), stop=(k == NKT - 1))
                        return p
                    sc = 0.015625 if (DR and l == 0) else 1.0

                    # --- phase A: cell,o gates (PSUM drained early) ---
                    # emit per chunk (c, o pairs) so sg/so/g for chunk j
                    # complete as early as possible
                    ps_c, ps_o = [], []
                    for j in range(NCHUNK):
                        ps_c.append(mm(3, j, "pc"))
                        ps_o.append(mm(2, j, "po"))
                    sgs, sos, gs = [], [], []
                    for j in range(NCHUNK):
                        sg = T2("sg", t, j, bufs=4)
                        nc.scalar.activation(sg[:], ps_c[j][:], AF.Sigmoid,
                                             bias=ba[:, 12 + j:13 + j],
                                             scale=sc)
                        so = T2("so", t, j, bufs=8)
                        nc.scalar.activation(so[:], ps_o[j][:], AF.Sigmoid,
                                             bias=ba[:, 8 + j:9 + j],
                                             scale=sc)
                        sgs.append(sg)
                        sos.append(so)
                    for j in range(NCHUNK):
                        # g = max(cell + bc, sig(cell)) fused; drains ps_c
                        g = T2("g", t, j, bufs=8)
                        if DR and l == 0:
                            cp5 = T2("cq", t, j)
                            nc.vector.tensor_scalar(cp5[:], ps_c[j][:], sc,
                                                    bc[:, j:j + 1],
                                                    ALU.mult, ALU.add)
                            nc.vector.tensor_tensor(g[:], cp5[:], sgs[j][:],
                                                    ALU.max)
                        else:
                            nc.vector.scalar_tensor_tensor(g[:], ps_c[j][:],
                                                           bc[:, j:j + 1],
                                                           sgs[j][:],
                                                           ALU.add, ALU.max)
                        gs.append(g)

                    # --- phase B: i,f gates ---
                    ps_i = [mm(0, j, "pi") for j in range(NCHUNK)]
                    ps_f = [mm(1, j, "pf") for j in range(NCHUNK)]
                    sfs, sis = [], []
                    for j in range(NCHUNK):
                        sf = T2("sf", t, j, bufs=8)
                        nc.scalar.activation(sf[:], ps_f[j][:], AF.Sigmoid,
                                             bias=ba[:, 4 + j:5 + j],
                                             scale=sc)
                        si = T2("si", t, j, bufs=4)
                        nc.scalar.activation(si[:], ps_i[j][:], AF.Sigmoid,
                                             bias=ba[:, j:j + 1],
                                             scale=sc)
                        sfs.append(sf)
                        sis.append(si)
                    ss_all = gp.tile([128, NCHUNK * TBLK], f32, tag="ss",
                                     name=f"ss{l}_{t}", bufs=2)
                    for j in range(NCHUNK):
                        nc.gpsimd.tensor_tensor(
                            ss_all[:, j * TBLK:(j + 1) * TBLK],
                            sfs[j][:], sis[j][:], ALU.add)

                    if pending is not None:
                        emit_tail(pending)
                    pending = (t, sfs, ss_all, gs, sos, h1own)
                emit_tail(pending)

    _split_multi_waits(nc)
    return nc


def _shard_inputs(x, W0, b0, W1, b1, c0_prev, c1_prev, mm_mode="fp8l1"):
    import ml_dtypes
    if mm_mode in ("bf16", "fp8l1"):
        mmdt = ml_dtypes.bfloat16
    else:
        mmdt = np.float32
    # fp8l1: layer-1 operands in TRN fp8e4 (max +-240); weights pre-scaled
    # by 64 so they sit in the normal range (the kernel rescales by 1/64
    # inside the activations).
    f8 = ml_dtypes.float8_e4m3
    xdt = [f8 if mm_mode == "fp8l1" else mmdt, mmdt]
    wscale = [np.float32(64.0) if mm_mode == "fp8l1" else np.float32(1.0),
              np.float32(1.0)]
    x = np.asarray(x, dtype=np.float32)
    in_maps = []
    xT = [np.ascontiguousarray(np.clip(x[b].T, -240, 240).astype(xdt[0]))
          for b in range(B)]
    per_layer = []
    for li, (W, bb) in enumerate(((W0, b0), (W1, b1))):
        W = np.asarray(W, dtype=np.float32)
        bb = np.asarray(bb, dtype=np.float32)
        halves = []
        for h in range(2):
            rows = np.concatenate(
                [q * D + h * HALF + np.arange(HALF) for q in range(4)])
            wt = np.ascontiguousarray(
                np.clip(W[rows, :].T * wscale[li], -240, 240)
                .astype(xdt[li]))  # (D, GCH)
            ba = np.ascontiguousarray(bb[rows].reshape(16, 128).T)  # (128,16)
            bc = np.ascontiguousarray(ba[:, 12:16] + np.float32(0.5))
            halves.append((wt, ba, bc))
        per_layer.append(halves)
    cps = []
    for cprev in (c0_prev, c1_prev):
        cprev = np.asarray(cprev, dtype=np.float32)
        halves = []
        for b in range(B):
            row = []
            for h in range(2):
                seg = cprev[b, 0, h * HALF:(h + 1) * HALF]
                row.append(np.ascontiguousarray(seg.reshape(4, 128).T))
            halves.append(row)
        cps.append(halves)
    for k in range(NCORES):
        b, h = k // 2, k % 2
        m = {"xT": xT[b]}
        for l in range(2):
            wt, ba, bc = per_layer[l][h]
            m[f"w{l}t"] = wt
            m[f"b{l}a"] = ba
            m[f"b{l}c"] = bc
            m[f"cp{l}"] = cps[l][b][h]
        in_maps.append(m)
    return in_maps


MM_MODE = os.environ.get("MINLSTM_MM_MODE", "fp8l1")


def _get_nc():
    if "nc" not in _CACHE:
        _CACHE["nc"] = _build_nc(mm_mode=MM_MODE)
    return _CACHE["nc"]


def kernel(x, W0, b0, W1, b1, c0_prev, c1_prev):
    from concourse.bass_utils import run_bass_kernel_spmd

    nc = _get_nc()
    in_maps = _shard_inputs(x, W0, b0, W1, b1, c0_prev, c1_prev, MM_MODE)
    res = run_bass_kernel_spmd(nc, in_maps, list(range(NCORES)))
    out = np.empty((B, S, D), dtype=np.float32)
    for k in range(NCORES):
        b, h = k // 2, k % 2
        out[b, :, h * HALF:(h + 1) * HALF] = res.results[k]["h2t"].T
    return out


# revision 33
# speedup vs baseline: 1.3854x; 1.0120x over previous
"""minLSTM (2-layer, B=4, S=4096, D=1024) on 8 Trainium2 NeuronCores.

Sharding: core k -> (batch b = k//2, channel half h = k%2).
Each core computes all 4096 timesteps for its batch and its 512 channels.

Math (exact rewrite of the reference; gates stay well inside +-10 for
these input scales so the clamp is a no-op):
  f' = sig(f)/(sig(f)+sig(i)),  i' = 1 - f'
  g  = max(cell + 0.5, sig(cell))
  c_t = f' c_{t-1} + i' g_t
  h   = sig(o) * c

GEMMs (default mode "fp8l1"): layer-1 gates in fp8e4 with
perf_mode=DoubleRow (2 contraction rows per PE cell, ~1.25x over bf16;
weights pre-scaled by 64 on the host, activations rescale by 1/64),
layer-2 in bf16 (fp8 there pushes the error past the 2e-2 gate).
Measured rel err 1.28e-2 vs the fp32 reference.

The 1/(sig(f)+sig(i)) reciprocal runs on the ACT engine's reciprocal
table (~1.2e-5 rel err) as ONE wide op per token block, so the walrus
scheduler cannot scatter it between sigmoids (each occurrence would
cost a 1.3us act-table load).  The scan value term is
btn = (f'-1)*g = -i'*g via one fused scalar_tensor_tensor, undone by
tensor_tensor_scan(mult, subtract); no division anywhere on the DVE.

Engine split per [128 x 512] tile:
  ACT : sig(cell), sig(o) | sig(f), sig(i) | recip(s)  (batched so the
        act table switches exactly twice per token block)
  DVE : g = max(cell + bc, sg)  (fused stt, PSUM read; via cp5 in fp8)
        btn = (a - 1) * g       (fused stt)
        c = scan(a, btn)
  Pool: ssum = sf+si, a = sf*r, h = so*c  (SBUF-only tensor_tensor)
The PE runs gates cell,o for all 4 chunks first, then i,f — every PSUM
bank is drained by an early consumer and the 8 banks cover the
c/o/i/f x 4-chunk working set with double buffering per tag.  Each
block's reciprocal/scan/store tail is emitted one block late so the
act-table switch never delays the next block's sigmoid phase; the
sf/so/g rings are sized (bufs=8) for that extended lifetime.

Both layers' weights prefetch up front on the Activation/GpSimd DMA
queues.  Between the layers, channel-half pairs exchange h1 (bf16) via
pairwise AllGather collectives, one per 512-token block, overlapped
with compute.

Self-contained: hardcodes shapes; only imports the system concourse repo.
"""
import os
import sys

if '/opt/trn_rl_repo' not in sys.path:
    sys.path.insert(0, '/opt/trn_rl_repo')

import numpy as np

B, S, D = 4, 4096, 1024
NCORES = 8
HALF = D // 2           # channels per core: 512
NCHUNK = HALF // 128    # 4 partition chunks of 128 channels
NKT = D // 128          # 8 contraction k-tiles
TBLK = 512              # token block
NBLK = S // TBLK        # 8 token blocks
GCH = 4 * HALF          # gate channels per core: 2048

_CACHE = {}


def _split_multi_waits(nc):
    """This walrus build rejects >1 sync wait per instruction. Hoist extra
    waits onto same-engine NoOps inserted just before; engine-queue program
    order makes this semantically identical."""
    from concourse import mybir
    n = 0
    for fn in nc.m.functions:
        for blk in fn.blocks:
            insts = list(blk.instructions)
            new = []
            changed = False
            for inst in insts:
                si = inst.sync_info
                ow = list(si.on_wait) if si is not None and si.on_wait else []
                if len(ow) > 1:
                    changed = True
                    for w in ow[:-1]:
                        n += 1
                        nop = mybir.InstNoOp(name=f"I-wsplit-{n}", ins=[], outs=[])
                        nop.engine = inst.engine
                        nop.sync_info = mybir.SyncInfo(on_wait=[w], on_update=[])
                        new.append(nop)
                    si.on_wait = [ow[-1]]
                new.append(inst)
            if changed:
                blk.instructions = new
    return n


def _build_nc(mm_mode="fp8l1", sim_local=False):
    import concourse.bass as bass
    import concourse.mybir as mybir
    import concourse.tile as tile

    f32 = mybir.dt.float32
    f8 = mybir.dt.float8e4
    DR = mm_mode == "fp8l1"  # layer-1 fp8 DoubleRow, layer-2 bf16
    fmm = {"f32r": mybir.dt.float32r, "f32": f32, "bf16": mybir.dt.bfloat16,
           "fp8l1": mybir.dt.bfloat16}[mm_mode]
    lmm = [f8 if DR else fmm, fmm]   # per-layer matmul dtype
    fh1 = mybir.dt.bfloat16 if mm_mode in ("bf16", "fp8l1") else f32
    PM = mybir.MatmulPerfMode
    AF = mybir.ActivationFunctionType
    ALU = mybir.AluOpType

    nc = bass.Bass("TRN2", target_bir_lowering=False, debug=False,
                   num_devices=NCORES)

    xT_d = nc.dram_tensor("xT", [D, S], lmm[0], kind="ExternalInput").ap()
    w_d = [nc.dram_tensor(f"w{l}t", [D, GCH], lmm[l], kind="ExternalInput").ap()
           for l in range(2)]
    ba_d = [nc.dram_tensor(f"b{l}a", [128, 16], f32, kind="ExternalInput").ap()
            for l in range(2)]
    bc_d = [nc.dram_tensor(f"b{l}c", [128, 4], f32, kind="ExternalInput").ap()
            for l in range(2)]
    cp_d = [nc.dram_tensor(f"cp{l}", [128, 4], f32, kind="ExternalInput").ap()
            for l in range(2)]
    h2t_d = nc.dram_tensor("h2t", [HALF, S], f32, kind="ExternalOutput").ap()

    with tile.TileContext(nc) as tc:
        with tc.tile_pool(name="wp", bufs=2) as wp, \
             tc.tile_pool(name="xkp", bufs=2) as xkp, \
             tc.tile_pool(name="gp", bufs=2) as gp, \
             tc.tile_pool(name="cp", bufs=1) as cpool, \
             tc.tile_pool(name="psum", bufs=2, space="PSUM") as psum, \
             tc.tile_pool(name="dstage", bufs=2, space="DRAM") as dstage, \
             tc.tile_pool(name="dfull", bufs=8, space="DRAM") as dfull:

            # h1 gathered blocks must persist through layer 2: 8 live tiles
            h1f = [dfull.tile([D, TBLK], fh1, tag="h1f", name=f"h1f{t}")
                   for t in range(NBLK)]

            # Prefetch BOTH layers' weights up front on queues other than
            # Sync (which carries the x loads the first matmuls wait on):
            # layer-1 on the Activation queue, layer-2 on GpSimd.  The DMA
            # engines drain all three descriptor streams concurrently.
            w_ks_all = []
            for l in range(2):
                w_ks = []
                eng = nc.gpsimd
                # In DR mode the two layers use distinct tags, so each tag
                # only ever holds one tile: bufs=1 (else SBUF overflows).
                wb = 1 if DR else 2
                if DR and l == 0:
                    for k4 in range(NKT // 2):
                        wk = wp.tile([128, 2, GCH], f8, tag=f"Wq{k4}",
                                     name=f"w{l}_{k4}", bufs=wb)
                        eng.dma_start(wk[:],
                                      w_d[l][k4 * 256:(k4 + 1) * 256, :])
                        w_ks.append(wk)
                else:
                    for k in range(NKT):
                        wk = wp.tile([128, GCH], lmm[l], tag=f"Wk{k}",
                                     name=f"w{l}_{k}", bufs=wb)
                        eng.dma_start(wk[:], w_d[l][k * 128:(k + 1) * 128, :])
                        w_ks.append(wk)
                w_ks_all.append(w_ks)

            for l in range(2):
                w_ks = w_ks_all[l]
                ba = cpool.tile([128, 16], f32, tag=f"ba{l}", name=f"ba{l}")
                nc.sync.dma_start(ba[:], ba_d[l][:])
                bc = cpool.tile([128, 4], f32, tag=f"bc{l}", name=f"bc{l}")
                nc.sync.dma_start(bc[:], bc_d[l][:])
                cp = cpool.tile([128, 4], f32, tag=f"cp{l}", name=f"cp{l}")
                nc.sync.dma_start(cp[:], cp_d[l][:])

                carry = [None] * NCHUNK

                def act_recip(out, in_):
                    # The act-table reciprocal measures ~1.2e-5 max rel err
                    # on (9e-5, 2] — emit InstActivation directly since the
                    # bass wrapper refuses Reciprocal.
                    se = nc.scalar
                    se.add_instruction(mybir.InstActivation(
                        name=nc.get_next_instruction_name(),
                        func=AF.Reciprocal,
                        ins=[se.lower_ap(in_),
                             mybir.ImmediateValue(dtype=f32, value=0.0),
                             mybir.ImmediateValue(dtype=f32, value=1.0),
                             mybir.ImmediateValue(dtype=f32, value=0.0)],
                        outs=[se.lower_ap(out)],
                    ))

                def emit_tail(st):
                    """Finish block st: r = 1/s (act table phase), a, btn,
                    scan, h, store + collective.  Emitted one block late so
                    the act-table switch never delays the next block's
                    sigmoid phase (whose DVE g-op gates PSUM bank reuse).
                    The reciprocal is ONE wide op over all 4 chunks so the
                    walrus scheduler cannot scatter it between sigmoids
                    (which would add an act-table load per occurrence)."""
                    t, sfs, ss_all, gs, sos, h1own = st
                    r_all = gp.tile([128, NCHUNK * TBLK], f32, tag="r",
                                    name=f"r{l}_{t}", bufs=2)
                    rs = [r_all[:, j * TBLK:(j + 1) * TBLK]
                          for j in range(NCHUNK)]
                    act_recip(r_all[:], ss_all[:])
                    for j in range(NCHUNK):
                        a = T2("a", t, j)
                        nc.gpsimd.tensor_tensor(a[:], sfs[j][:], rs[j],
                                                ALU.mult)
                        btn = T2("bt", t, j)
                        nc.vector.scalar_tensor_tensor(btn[:], a[:], 1.0,
                                                       gs[j][:], ALU.subtract,
                                                       ALU.mult)
                        c = T2(f"c{j}", t, j)
                        init = cp[:, j:j + 1] if t == 0 else carry[j]
                        nc.vector.tensor_tensor_scan(c[:], a[:], btn[:],
                                                     init, ALU.mult,
                                                     ALU.subtract)
                        carry[j] = c[:, TBLK - 1:TBLK]
                        hdt = fh1 if l == 0 else f32
                        h = T2(f"h{l}", t, j, hdt)
                        nc.gpsimd.tensor_tensor(h[:], sos[j][:], c[:], ALU.mult)

                        if l == 0:
                            nc.sync.dma_start(
                                h1own[j * 128:(j + 1) * 128, :], h[:])
                        else:
                            nc.sync.dma_start(
                                h2t_d[j * 128:(j + 1) * 128,
                                      t * TBLK:(t + 1) * TBLK], h[:])

                    if l == 0:
                        if sim_local:
                            nc.sync.dma_start(h1f[t][0:HALF, :], h1own[:])
                            nc.sync.dma_start(h1f[t][HALF:D, :], h1own[:])
                        else:
                            nc.gpsimd.collective_compute(
                                "AllGather", ALU.bypass,
                                replica_groups=[[0, 1], [2, 3], [4, 5], [6, 7]],
                                ins=[h1own.opt()],
                                outs=[h1f[t].opt()],
                            )

                def T2(nm, t, j, dt=f32, bufs=2):
                    return gp.tile([128, TBLK], dt, tag=nm,
                                   name=f"{nm}{l}_{t}_{j}", bufs=bufs)

                def W2(nm, t, dt=f32, bufs=2):
                    return gp.tile([128, NCHUNK * TBLK], dt, tag=nm,
                                   name=f"{nm}{l}_{t}", bufs=bufs)

                pending = None
                for t in range(NBLK):
                    xk_ks = []
                    if DR and l == 0:
                        for k4 in range(NKT // 2):
                            xkt = xkp.tile([128, 2, TBLK], f8, tag=f"xq{k4}",
                                           name=f"xq{l}_{t}_{k4}")
                            nc.sync.dma_start(
                                xkt[:], xT_d[k4 * 256:(k4 + 1) * 256,
                                             t * TBLK:(t + 1) * TBLK])
                            xk_ks.append(xkt)
                    else:
                        for k in range(NKT):
                            xkt = xkp.tile([128, TBLK], lmm[l], tag=f"xk{k}",
                                           name=f"xk{l}_{t}_{k}")
                            if l == 0:
                                srcap = xT_d[k * 128:(k + 1) * 128,
                                             t * TBLK:(t + 1) * TBLK]
                            else:
                                srcap = h1f[t][k * 128:(k + 1) * 128, :]
                            nc.sync.dma_start(
                                xkt[:],
                                srcap if srcap.dtype == lmm[l]
                                else srcap.bitcast(lmm[l]))
                            xk_ks.append(xkt)

                    if l == 0:
                        h1own = dstage.tile([HALF, TBLK], fh1, tag="h1own",
                                            name=f"h1own{t}")
                    else:
                        h1own = None

                    def mm(qi, j, tag):
                        ct = qi * NCHUNK + j
                        p = psum.tile([128, TBLK], f32, tag=tag,
                                      name=f"ps{qi}_{l}_{t}_{j}")
                        if DR and l == 0:
                            for k4 in range(NKT // 2):
                                nc.tensor.matmul(
                                    p[:],
                                    w_ks[k4][:, :, ct * 128:(ct + 1) * 128],
                                    xk_ks[k4][:],
                                    start=(k4 == 0), stop=(k4 == NKT // 2 - 1),
                                    perf_mode=PM.DoubleRow)
                        else:
                            for k in range(NKT):
                                nc.tensor.matmul(
                                    p[:],
                                    w_ks[k][:, ct * 128:(ct + 1) * 128],
                                    xk_ks[k][:],
                                    start=(k == 0), stop=(k == NKT - 1))
                        return p
                    sc = 0.015625 if (DR and l == 0) else 1.0

                    # --- phase A: cell,o gates (PSUM drained early) ---
                    # emit per chunk (c, o pairs) so sg/so/g for chunk j
                    # complete as early as possible
                    ps_c, ps_o = [], []
                    for j in range(NCHUNK):
                        ps_c.append(mm(3, j, "pc"))
                        ps_o.append(mm(2, j, "po"))
                    sgs, sos, gs = [], [], []
                    for j in range(NCHUNK):
                        sg = T2("sg", t, j, bufs=4)
                        nc.scalar.activation(sg[:], ps_c[j][:], AF.Sigmoid,
                                             bias=ba[:, 12 + j:13 + j],
                                             scale=sc)
                        so = T2("so", t, j, bufs=8)
                        nc.scalar.activation(so[:], ps_o[j][:], AF.Sigmoid,
                                             bias=ba[:, 8 + j:9 + j],
                                             scale=sc)
                        sgs.append(sg)
                        sos.append(so)
                    for j in range(NCHUNK):
                        # g = max(cell + bc, sig(cell)) fused; drains ps_c
                        g = T2("g", t, j, bufs=8)
                        if DR and l == 0:
                            cp5 = T2("cq", t, j)
                            nc.vector.tensor_scalar(cp5[:], ps_c[j][:], sc,
                                                    bc[:, j:j + 1],
                                                    ALU.mult, ALU.add)
                            nc.vector.tensor_tensor(g[:], cp5[:], sgs[j][:],
                                                    ALU.max)
                        else:
                            nc.vector.scalar_tensor_tensor(g[:], ps_c[j][:],
                                                           bc[:, j:j + 1],
                                                           sgs[j][:],
                                                           ALU.add, ALU.max)
                        gs.append(g)

                    # --- phase B: i,f gates ---
                    # f first per chunk: sf is the first act emitted below
                    ps_i, ps_f = [None] * NCHUNK, [None] * NCHUNK
                    for j in range(NCHUNK):
                        ps_f[j] = mm(1, j, "pf")
                        ps_i[j] = mm(0, j, "pi")
                    sfs, sis = [], []
                    for j in range(NCHUNK):
                        sf = T2("sf", t, j, bufs=8)
                        nc.scalar.activation(sf[:], ps_f[j][:], AF.Sigmoid,
                                             bias=ba[:, 4 + j:5 + j],
                                             scale=sc)
                        si = T2("si", t, j, bufs=4)
                        nc.scalar.activation(si[:], ps_i[j][:], AF.Sigmoid,
                                             bias=ba[:, j:j + 1],
                                             scale=sc)
                        sfs.append(sf)
                        sis.append(si)
                    ss_all = gp.tile([128, NCHUNK * TBLK], f32, tag="ss",
                                     name=f"ss{l}_{t}", bufs=2)
                    for j in range(NCHUNK):
                        nc.gpsimd.tensor_tensor(
                            ss_all[:, j * TBLK:(j + 1) * TBLK],
                            sfs[j][:], sis[j][:], ALU.add)

                    if pending is not None:
                        emit_tail(pending)
                    pending = (t, sfs, ss_all, gs, sos, h1own)
                emit_tail(pending)

    _split_multi_waits(nc)
    return nc


def _shard_inputs(x, W0, b0, W1, b1, c0_prev, c1_prev, mm_mode="fp8l1"):
    import ml_dtypes
    if mm_mode in ("bf16", "fp8l1"):
        mmdt = ml_dtypes.bfloat16
    else:
        mmdt = np.float32
    # fp8l1: layer-1 operands in TRN fp8e4 (max +-240); weights pre-scaled
    # by 64 so they sit in the normal range (the kernel rescales by 1/64
    # inside the activations).
    f8 = ml_dtypes.float8_e4m3
    xdt = [f8 if mm_mode == "fp8l1" else mmdt, mmdt]
    wscale = [np.float32(64.0) if mm_mode == "fp8l1" else np.float32(1.0),
              np.float32(1.0)]
    x = np.asarray(x, dtype=np.float32)
    in_maps = []
    xT = [np.ascontiguousarray(np.clip(x[b].T, -240, 240).astype(xdt[0]))
          for b in range(B)]
    per_layer = []
    for li, (W, bb) in enumerate(((W0, b0), (W1, b1))):
        W = np.asarray(W, dtype=np.float32)
        bb = np.asarray(bb, dtype=np.float32)
        halves = []
        for h in range(2):
            rows = np.concatenate(
                [q * D + h * HALF + np.arange(HALF) for q in range(4)])
            wt = np.ascontiguousarray(
                np.clip(W[rows, :].T * wscale[li], -240, 240)
                .astype(xdt[li]))  # (D, GCH)
            ba = np.ascontiguousarray(bb[rows].reshape(16, 128).T)  # (128,16)
            bc = np.ascontiguousarray(ba[:, 12:16] + np.float32(0.5))
            halves.append((wt, ba, bc))
        per_layer.append(halves)
    cps = []
    for cprev in (c0_prev, c1_prev):
        cprev = np.asarray(cprev, dtype=np.float32)
        halves = []
        for b in range(B):
            row = []
            for h in range(2):
                seg = cprev[b, 0, h * HALF:(h + 1) * HALF]
                row.append(np.ascontiguousarray(seg.reshape(4, 128).T))
            halves.append(row)
        cps.append(halves)
    for k in range(NCORES):
        b, h = k // 2, k % 2
        m = {"xT": xT[b]}
        for l in range(2):
            wt, ba, bc = per_layer[l][h]
            m[f"w{l}t"] = wt
            m[f"b{l}a"] = ba
            m[f"b{l}c"] = bc
            m[f"cp{l}"] = cps[l][b][h]
        in_maps.append(m)
    return in_maps


MM_MODE = os.environ.get("MINLSTM_MM_MODE", "fp8l1")


def _get_nc():
    if "nc" not in _CACHE:
        _CACHE["nc"] = _build_nc(mm_mode=MM_MODE)
    return _CACHE["nc"]


def kernel(x, W0, b0, W1, b1, c0_prev, c1_prev):
    from concourse.bass_utils import run_bass_kernel_spmd

    nc = _get_nc()
    in_maps = _shard_inputs(x, W0, b0, W1, b1, c0_prev, c1_prev, MM_MODE)
    res = run_bass_kernel_spmd(nc, in_maps, list(range(NCORES)))
    out = np.empty((B, S, D), dtype=np.float32)
    for k in range(NCORES):
        b, h = k // 2, k % 2
        out[b, :, h * HALF:(h + 1) * HALF] = res.results[k]["h2t"].T
    return out
